# revision 1
# baseline (speedup 1.0000x reference)
"""2-layer LSTM decoder Bass/Tile kernel for TRN2.

Per-core: B_local=128 batch rows (one SBUF partition width), H=512, 64 steps.
Data-parallel over 8 cores; weights replicated.

Layout: batch on partitions, gates on free dim ("Layout A").
  gates[b, 4H] = h @ W_hh.T + in_aug @ W_ih_aug.T
  matmul(out=gates_psum, lhsT=h.T chunks (from PE transposes), rhs=W.T chunks)
Gate column order is permuted to [i, f, o, g] so sigmoid covers a contiguous
[0:1536] span and tanh [1536:2048].
All matmuls run in float32r (TF32-like, 1 cycle/row at N>=512).
"""

import numpy as np
from contextlib import ExitStack

import concourse.bass as bass
import concourse.bacc as bacc
import concourse.mybir as mybir
import concourse.tile as tile
from concourse import masks

F32 = mybir.dt.float32
F32R = mybir.dt.float32r
AF = mybir.ActivationFunctionType
OP = mybir.AluOpType

P = 128          # batch rows per core / partition width
H = 512          # hidden
G = 2048         # 4*H gates
NT = 4           # n-tiles of 512 across gates
KC = 4           # K chunks of 128 across H
EXO = 8
ZD = 16
# gate reorder: source gate block g (pytorch i,f,g,o) -> dest block in [i,f,o,g]
NEWPOS = [0, 1, 3, 2]
PER_TILE = True  # per-tile matmul groups + per-tile ACT vs bulk emission


def r(ap):
    return ap.bitcast(F32R)


def f(ap):
    return ap.bitcast(F32)


def build_kernel(nc: bass.Bass, steps: int, repeat: int = 1):
    """Emit the full kernel (inputs/outputs declared on nc)."""
    S = steps
    # ---- DRAM params ----
    di = lambda name, shape: nc.dram_tensor(name, shape, F32, kind="ExternalInput").ap()
    y0 = di("y0", [P, 1])
    xf = di("x_future", [P, S * EXO])         # host passes reshaped (P, S*EXO)
    h00 = di("h00", [P, H])
    h01 = di("h01", [P, H])
    c00 = di("c00", [P, H])
    c01 = di("c01", [P, H])
    z = di("z", [P, ZD])
    Wih0 = di("W_ih0", [G, 9])
    Whh0 = di("W_hh0", [G, H])
    bih0 = di("b_ih0", [1, G])
    bhh0 = di("b_hh0", [1, G])
    Wih1 = di("W_ih1", [G, H])
    Whh1 = di("W_hh1", [G, H])
    bih1 = di("b_ih1", [1, G])
    bhh1 = di("b_hh1", [1, G])
    Wp = di("W_proj", [1, H])
    bp = di("b_proj", [1, 1])
    Wz = di("W_z", [9, ZD])
    bz = di("b_z", [1, 9])
    out_d = nc.dram_tensor("out", [S, P], F32, kind="ExternalOutput").ap()

    with tile.TileContext(nc) as tc, ExitStack() as ctx:
        emit(ctx, tc, nc, S, locals(), repeat)
    return nc


def emit(ctx, tc, nc, S, t_in, repeat=1):
    xf, y0, z = t_in["xf"], t_in["y0"], t_in["z"]
    h00, h01, c00, c01 = t_in["h00"], t_in["h01"], t_in["c00"], t_in["c01"]
    Wih0, Whh0, Wih1, Whh1 = t_in["Wih0"], t_in["Whh0"], t_in["Wih1"], t_in["Whh1"]
    bih0, bhh0, bih1, bhh1 = t_in["bih0"], t_in["bhh0"], t_in["bih1"], t_in["bhh1"]
    Wp, bp, Wz, bz = t_in["Wp"], t_in["bp"], t_in["Wz"], t_in["bz"]
    out_d = t_in["out_d"]

    # ---- pools ----
    const = ctx.enter_context(tc.tile_pool(name="const", bufs=1))
    state = ctx.enter_context(tc.tile_pool(name="state", bufs=2))
    gact = ctx.enter_context(tc.tile_pool(name="gact", bufs=2))
    yo = ctx.enter_context(tc.tile_pool(name="yo", bufs=3))
    ldtmp = ctx.enter_context(tc.tile_pool(name="ldtmp", bufs=2))
    gsig = ctx.enter_context(tc.tile_pool(name="gsig", bufs=2, space="PSUM"))
    gtan = ctx.enter_context(tc.tile_pool(name="gtan", bufs=1, space="PSUM"))
    tpp = ctx.enter_context(tc.tile_pool(name="tpp", bufs=1, space="PSUM"))

    # ---- persistent SBUF ----
    WhhT0 = const.tile([P, KC * G], F32R, name="WhhT0")
    WihT1 = const.tile([P, KC * G], F32R, name="WihT1")
    WhhT1 = const.tile([P, KC * G], F32R, name="WhhT1")
    # packA layout (free offsets):
    #   [0:S*128]                 p0-9   : inT_all (exo rows 0-7, ones row 8, y row 9)
    #   [S*128 : S*128+2048]      p0-9   : Wih0a10 (rhs for K=10 input matmul)
    #   [0:128]                   p32    : ones_row (lhsT for bias matmul)
    #   [128:2176]                p32    : b1row (rhs for bias matmul)
    #   [S*128+2048 : +4]         p0-127 : WpT  (proj lhsT chunks)
    #   then [1,128] rows: bp_row, ybias_row
    oWA = S * 128
    oWpT = oWA + G
    packA = const.tile([P, oWpT + 32], F32R, name="packA")
    # packA is read-only during the step loop (y rows live in rotating
    # tiles); exo rows at partitions 32-39, ones row at partition 40 so the
    # K=9 exo matmul has no y dependency; wcol/ones0 tucked under p0.
    inT_x = packA[32:41, 0 : S * 128]
    Wih0a9 = packA[32:41, oWA : oWA + G]
    wcol_row = packA[0:1, 0:G]
    ones0 = packA[0:1, G : G + 128]
    ones_row = packA[64:65, 0:128]
    b1row = packA[64:65, 128 : 128 + G]
    WpT = packA[:, oWpT : oWpT + 4]
    bp_row = const.tile([1, P], F32, name="bp_row")
    ybias_row = const.tile([1, P], F32, name="ybias_row")

    ident_f = const.tile([P, P], F32, name="ident_f")
    masks.make_identity(nc, ident_f[:])
    ident = const.tile([P, P], F32R, name="ident")
    nc.scalar.copy(ident[:], ident_f[:])
    idr = ident

    zb_sb = const.tile([P, 16], F32R, name="zb_sb")
    WzT = const.tile([16, 16], F32R, name="WzT")
    bz_sb = const.tile([1, 16], F32R, name="bz_sb")

    dma = nc.sync.dma_start
    ones_f = const.tile([1, P], F32, name="ones_f")
    nc.vector.memset(ones_f[:], 1.0)
    nc.scalar.copy(ones0, ones_f[:])

    # =========================== INIT ===========================
    # -- z bias: zb = z @ Wz.T + bz  ([P, 9]) --
    z_sb = ldtmp.tile([P, ZD], F32R, name="z_sb", tag="ldtmp")
    nc.gpsimd.dma_start(z_sb[:], z)
    zt_ps = tpp.tile([P, H], F32R, name="ztps", tag="tp")
    nc.tensor.matmul(zt_ps[0:ZD, 0:P], z_sb[:], idr[:], is_transpose=True)
    zT_sb = const.tile([16, P], F32R, name="zT_sb")
    nc.scalar.copy(zT_sb[:], zt_ps[0:ZD, 0:P])
    nc.gpsimd.dma_start(WzT[:, 0:9], Wz.rearrange("a b -> b a"))
    nc.gpsimd.dma_start(WzT[:, 9:16], Wz.rearrange("a b -> b a")[:, 2:9])
    nc.gpsimd.dma_start(bz_sb[:, 0:9], bz)
    nc.gpsimd.dma_start(bz_sb[:, 9:16], bz[:, 2:9])
    zb_ps = tpp.tile([P, H], F32, name="zbps", tag="tp")
    nc.tensor.matmul(zb_ps[:, 0:16], zT_sb[:], WzT[:], start=True, stop=False)
    nc.tensor.matmul(zb_ps[:, 0:16], ones0, bz_sb[:], start=False, stop=True)
    nc.scalar.copy(zb_sb[:, 0:9], zb_ps[:, 0:9])  # rounds to f32r

    # -- x_future + z_bias[:,1:9]; transpose into inT_all rows 0..7 --
    x_sb = ldtmp.tile([P, S * EXO], F32R, name="x_sb", tag="xsb")
    nc.gpsimd.dma_start(x_sb[:], xf)
    x3 = x_sb.rearrange("p (t e) -> p t e", e=EXO)
    zb3 = zb_sb[:, 1:9].unsqueeze(1).broadcast_to((P, S, EXO))
    nc.vector.tensor_tensor(x3, x3, zb3, op=OP.add)
    tpb = P // EXO  # 16 t-values per 128-wide transpose block
    xt_sb = ldtmp.tile([P, max(P, S * EXO)], F32R, name="xt_sb", tag="xsb")
    if S % tpb == 0:
        nxb = (S * EXO) // P  # number of 128-wide transpose blocks (S=64 -> 4)
        for j in range(nxb):
            xt_ps = tpp.tile([P, H], F32R, name="xtps", tag="tp")
            nc.tensor.matmul(
                xt_ps[:, 0:P], x_sb[:, j * P : (j + 1) * P], idr[:],
                is_transpose=True, start=True, stop=True,
            )
            nc.scalar.copy(xt_sb[:, j * P : (j + 1) * P], xt_ps[:, 0:P])
        src4 = xt_sb[:, 0 : S * EXO].rearrange("p (j b) -> p j b", b=P)
        dst4 = inT_x[0:8, :].rearrange("p (j tm b) -> p tm j b", tm=tpb, b=P)
        for tm in range(tpb):
            dma(dst4[:, tm], src4[tm * EXO : (tm + 1) * EXO, :])
    else:
        assert S < tpb
        xt_ps = tpp.tile([P, H], F32R, name="xtps", tag="tp")
        nc.tensor.matmul(
            xt_ps[0 : S * EXO, 0:P], x_sb[:], idr[:],
            is_transpose=True, start=True, stop=True,
        )
        nc.scalar.copy(xt_sb[0 : S * EXO, 0:P], xt_ps[0 : S * EXO, 0:P])
        for tm in range(S):
            dma(
                inT_x[0:8, tm * P : (tm + 1) * P],
                xt_sb[tm * EXO : (tm + 1) * EXO, 0:P],
            )
    # ones row (partition 40): copy from ones0 via DMA
    for s in range(S):
        dma(inT_x[8:9, s * P : (s + 1) * P], ones0)

    # -- y row for t=0: y0.T + zb[:,0].T  (row 0, partition 0) --
    zb0t_ps = tpp.tile([P, H], F32R, name="zb0tps", tag="tp")
    nc.tensor.matmul(zb0t_ps[0:1, 0:P], zb_sb[:, 0:1], idr[:], is_transpose=True)
    zb0row = const.tile([1, P], F32, name="zb0row")
    nc.scalar.copy(zb0row[:], f(zb0t_ps)[0:1, 0:P])
    y0_sb = ldtmp.tile([P, 1], F32R, name="y0sb", tag="y0sb")
    nc.gpsimd.dma_start(y0_sb[:], y0)


    # -- bp_row / ybias_row --
    bp_sb = const.tile([1, 1], F32, name="bp_sb")
    dma(bp_sb[:], bp)
    nc.vector.tensor_copy(bp_row[:], bp_sb[0:1, 0:1].broadcast_to((1, P)))
    nc.vector.tensor_tensor(ybias_row[:], zb0row[:], bp_row[:], op=OP.add)

    # -- ones_row (partition 32): copy from ones0 via DMA --
    dma(ones_row, ones0)

    # -- WpT: WpT[p, k] = Wp[0, k*128+p] --
    nc.gpsimd.dma_start(WpT, Wp.rearrange("o (k p) -> p (o k)", p=P))

    # -- Wih0a10: row 0 = W_ih0[:,0].T (y col), rows 1-8 = W_ih0[:,1:9].T,
    #    row 9 = b_ih0+b_hh0 (columns gate-reordered) --
    for g in range(4):
        cdst = NEWPOS[g] * H
        src = Wih0[g * H : (g + 1) * H, 1:9].rearrange("a b -> b a")
        nc.gpsimd.dma_start(Wih0a9[0:8, cdst : cdst + H], src)
        src0 = Wih0[g * H : (g + 1) * H, 0:1].rearrange("a b -> b a")
        nc.gpsimd.dma_start(wcol_row[:, cdst : cdst + H], src0)

    # bias rows: load both bias vectors into gate-permuted [4, 512] staging
    # tiles (partition g' = NEWPOS[g]), add, then cast-DMA to the f32r row.
    def bias_row(b_a, b_b, dst_row):
        t1 = ldtmp.tile([4, H], F32, name="bs1", tag="ldtmp")
        t2 = ldtmp.tile([4, H], F32, name="bs2", tag="ldtmp")
        for g in range(4):
            dma(t1[NEWPOS[g] : NEWPOS[g] + 1, :], b_a[:, g * H : (g + 1) * H])
            dma(t2[NEWPOS[g] : NEWPOS[g] + 1, :], b_b[:, g * H : (g + 1) * H])
        nc.vector.tensor_tensor(t1[:], t1[:], t2[:], op=OP.add)
        for b in range(4):
            nc.gpsimd.dma_start(
                dst_row[:, b * H : (b + 1) * H], t1[b : b + 1, :]
            )

    bias_row(bih0, bhh0, Wih0a9[8:9, :])
    bias_row(bih1, bhh1, b1row)

    # -- big transposed weights: W [G, H] -> WT[p, k*G + dstblk*128 + c] --
    def build_WT(Wsrc, WT):
        WT4 = WT.rearrange("p (k mb mc) -> p k mb mc", k=KC, mc=P)
        for rr in range(16):
            wt = ldtmp.tile([P, H], F32R, name="wld", tag="ldtmp")
            nc.gpsimd.dma_start(wt[:], Wsrc[rr * P : (rr + 1) * P, :])
            tp_t = tpp.tile([P, H], F32R, name="wtps", tag="tp")
            for k in range(KC):
                nc.tensor.matmul(
                    tp_t[:, k * P : (k + 1) * P], wt[:, k * P : (k + 1) * P], idr[:],
                    is_transpose=True, start=(k == 0), stop=(k == KC - 1),
                )
            dstblk = NEWPOS[rr // 4] * 4 + (rr % 4)
            nc.scalar.copy(
                WT4[:, :, dstblk, :],
                tp_t.rearrange("p (k c) -> p k c", k=KC),
            )

    build_WT(Whh0, WhhT0)
    build_WT(Wih1, WihT1)
    build_WT(Whh1, WhhT1)

    # -- initial states + transposes --
    def load_state(src, tag):
        t = state.tile([P, H], F32R, name=tag, tag=tag)
        nc.gpsimd.dma_start(t[:], src)
        return t

    def transpose_state(h, tag):
        tp_t = tpp.tile([P, H], F32R, name=tag + "ps", tag="tp")
        for k in range(KC):
            nc.tensor.matmul(
                tp_t[:, k * P : (k + 1) * P], h[:, k * P : (k + 1) * P], idr[:],
                is_transpose=True, start=(k == 0), stop=(k == KC - 1),
            )
        hT = state.tile([P, H], F32R, name=tag, tag=tag)
        nc.scalar.copy(hT[:], tp_t[:])
        return hT

    # =========================== STEP LOOP ===========================
    # Gate-tile processing order: tanh tile (g) first so the DVE chain can
    # start while later sigma tiles are still in the matmul stream.
    N_ORDER = [3, 0, 1, 2]

    def lstm_tail(sg, tg, c_prev, ctag, ttag):
        """c' = sg[f]*c + sg[i]*tg ; hT = (sg[o].T) * tanh(c').T directly in
        transposed layout (no untransposed h is ever materialized)."""
        tmp = gact.tile([P, H], F32, name="tmp", tag="tmp")
        nc.vector.tensor_tensor(tmp[:], sg[0][:], tg[:], op=OP.mult)
        c_n = state.tile([P, H], F32R, name=ctag, tag=ctag)
        nc.vector.tensor_tensor(c_n[:], sg[1][:], c_prev[:], op=OP.mult)
        nc.vector.tensor_tensor(c_n[:], c_n[:], tmp[:], op=OP.add)
        # tanh(c') transposes+evicts early (thc lands before sigma_o);
        # sigma_o transposes stay in PSUM and feed the hT multiply directly.
        thc = gact.tile([P, H], F32R, name="thc", tag="thc")
        nc.scalar.activation(thc[:], c_n[:], AF.Tanh)
        th_tp = gsig.tile([P, H], F32R, name="thtp", tag="gsig")
        for k in range(KC):
            s = slice(k * P, (k + 1) * P)
            nc.tensor.matmul(
                th_tp[:, s], thc[:, s], idr[:],
                is_transpose=True, start=(k == 0), stop=(k == KC - 1),
            )
        thT = gact.tile([P, H], F32R, name="thT", tag="soT")
        nc.scalar.copy(thT[:], th_tp[:])
        so_tp = tpp.tile([P, H], F32R, name="sotp", tag="tp")
        for k in range(KC):
            s = slice(k * P, (k + 1) * P)
            nc.tensor.matmul(
                so_tp[:, s], sg[2][:, s], idr[:],
                is_transpose=True, start=(k == 0), stop=(k == KC - 1),
            )
        hT = state.tile([P, H], F32R, name=ttag, tag=ttag)
        nc.vector.tensor_tensor(hT[:], thT[:], so_tp[:], op=OP.mult)
        return c_n, hT

    for rep in range(repeat):
        h0_c = load_state(h00, "h0")
        h1_c = load_state(h01, "h1")
        c0_c = load_state(c00, "c0")
        c1_c = load_state(c01, "c1")
        h0T_c = transpose_state(h0_c, "h0T")
        h1T_c = transpose_state(h1_c, "h1T")
        zz_ps = tpp.tile([P, H], F32R, name="y0tps2", tag="tp")
        nc.tensor.matmul(zz_ps[0:1, 0:P], y0_sb[:], idr[:], is_transpose=True)
        yrow_c = yo.tile([1, P], F32R, name="yrow", tag="yrow")
        nc.vector.tensor_tensor(yrow_c[:], f(zz_ps)[0:1, 0:P], zb0row[:], op=OP.add)

        for t in range(S):
            # ---- layer 0 gates: per-tile [4 hidden MMs, K=10 input MM] + ACT ----
            gs0 = gsig.tile([P, 3 * H], F32, name="gs0", tag="gsig")
            gt0 = gtan.tile([P, H], F32, name="gt0", tag="gtan")
            g0 = lambda n: gs0[:, n * H : (n + 1) * H] if n < 3 else gt0[:]
            lx = inT_x[:, t * P : (t + 1) * P]
            ly = yrow_c[:]
            sg = [None] * 3
            tg = None
            if PER_TILE:
                for n in N_ORDER:
                    nc.tensor.matmul(
                        g0(n), lx, Wih0a9[:, n * H : (n + 1) * H], start=True, stop=False
                    )
                    for k in range(KC):
                        nc.tensor.matmul(
                            g0(n), h0T_c[:, k * P : (k + 1) * P],
                            WhhT0[:, k * G + n * H : k * G + (n + 1) * H],
                            start=False, stop=False,
                        )
                    nc.tensor.matmul(
                        g0(n), ly, wcol_row[0:1, n * H : (n + 1) * H], start=False, stop=True
                    )
                    if n == 3:
                        tg = gact.tile([P, H], F32, name="tg", tag="tg")
                        nc.scalar.activation(tg[:], gt0[:], AF.Tanh)
                    else:
                        dt = F32R if n == 2 else F32
                        sg[n] = gact.tile([P, H], dt, name=f"sg{n}", tag=f"sg{n}")
                        nc.scalar.activation(sg[n][:], gs0[:, n * H : (n + 1) * H], AF.Sigmoid)
            else:
                for n in range(NT):
                    nc.tensor.matmul(
                        g0(n), lx, Wih0a9[:, n * H : (n + 1) * H], start=True, stop=False
                    )
                for k in range(KC):
                    for n in range(NT):
                        nc.tensor.matmul(
                            g0(n), h0T_c[:, k * P : (k + 1) * P],
                            WhhT0[:, k * G + n * H : k * G + (n + 1) * H],
                            start=False, stop=False,
                        )
                for n in range(NT):
                    nc.tensor.matmul(
                        g0(n), ly, wcol_row[0:1, n * H : (n + 1) * H], start=False, stop=True
                    )
                tg = gact.tile([P, H], F32, name="tg", tag="tg")
                nc.scalar.activation(tg[:], gt0[:], AF.Tanh)
                for n in range(3):
                    dt = F32R if n == 2 else F32
                    sg[n] = gact.tile([P, H], dt, name=f"sg{n}", tag=f"sg{n}")
                    nc.scalar.activation(sg[n][:], gs0[:, n * H : (n + 1) * H], AF.Sigmoid)

            c0_c, h0T_n = lstm_tail(sg, tg, c0_c, "c0", "h0T")
            h0T_c = h0T_n

            # ---- layer 1 gates: per-tile [4 h1 MMs, 4 h0 MMs, bias MM] + ACT ----
            gs1 = gsig.tile([P, 3 * H], F32, name="gs1", tag="gsig")
            gt1 = gtan.tile([P, H], F32, name="gt1", tag="gtan")
            g1 = lambda n: gs1[:, n * H : (n + 1) * H] if n < 3 else gt1[:]
            sg1 = [None] * 3
            tg1 = None
            if PER_TILE:
                for n in N_ORDER:
                    for k in range(KC):
                        nc.tensor.matmul(
                            g1(n), h1T_c[:, k * P : (k + 1) * P],
                            WhhT1[:, k * G + n * H : k * G + (n + 1) * H],
                            start=(k == 0), stop=False,
                        )
                    for k in range(KC):
                        nc.tensor.matmul(
                            g1(n), h0T_n[:, k * P : (k + 1) * P],
                            WihT1[:, k * G + n * H : k * G + (n + 1) * H],
                            start=False, stop=False,
                        )
                    nc.tensor.matmul(
                        g1(n), ones_row, b1row[:, n * H : (n + 1) * H], start=False, stop=True
                    )
                    if n == 3:
                        tg1 = gact.tile([P, H], F32, name="tg", tag="tg")
                        nc.scalar.activation(tg1[:], gt1[:], AF.Tanh)
                    else:
                        dt = F32R if n == 2 else F32
                        sg1[n] = gact.tile([P, H], dt, name=f"sg{n}", tag=f"sg{n}")
                        nc.scalar.activation(sg1[n][:], gs1[:, n * H : (n + 1) * H], AF.Sigmoid)
            else:
                for k in range(KC):
                    for n in range(NT):
                        nc.tensor.matmul(
                            g1(n), h1T_c[:, k * P : (k + 1) * P],
                            WhhT1[:, k * G + n * H : k * G + (n + 1) * H],
                            start=(k == 0), stop=False,
                        )
                for k in range(KC):
                    for n in range(NT):
                        nc.tensor.matmul(
                            g1(n), h0T_n[:, k * P : (k + 1) * P],
                            WihT1[:, k * G + n * H : k * G + (n + 1) * H],
                            start=False, stop=False,
                        )
                for n in range(NT):
                    nc.tensor.matmul(
                        g1(n), ones_row, b1row[:, n * H : (n + 1) * H], start=False, stop=True
                    )
                tg1 = gact.tile([P, H], F32, name="tg", tag="tg")
                nc.scalar.activation(tg1[:], gt1[:], AF.Tanh)
                for n in range(3):
                    dt = F32R if n == 2 else F32
                    sg1[n] = gact.tile([P, H], dt, name=f"sg{n}", tag=f"sg{n}")
                    nc.scalar.activation(sg1[n][:], gs1[:, n * H : (n + 1) * H], AF.Sigmoid)

            c1_c, h1T_n = lstm_tail(sg1, tg1, c1_c, "c1", "h1T")
            h1T_c = h1T_n

            # ---- projection: yT = Wp @ h1.T  ([1, 128]) ----
            ytp = tpp.tile([P, H], F32, name="ytp", tag="tp")
            for k in range(KC):
                nc.tensor.matmul(
                    ytp[0:1, 0:P], WpT[:, k : k + 1], h1T_n[:, k * P : (k + 1) * P],
                    start=(k == 0), stop=(k == KC - 1),
                )
            y_pure = yo.tile([1, P], F32, name="yout", tag="yout")
            nc.vector.tensor_tensor(y_pure[:], ytp[0:1, 0:P], bp_row[:], op=OP.add)
            dma(out_d[t : t + 1, :], y_pure[:])
            if t + 1 < S:
                yrow_c = yo.tile([1, P], F32R, name="yrow", tag="yrow")
                nc.vector.tensor_tensor(
                    yrow_c[:], ytp[0:1, 0:P], ybias_row[:], op=OP.add
                )


def make_nc(steps: int, repeat: int = 1):
    nc = bacc.Bacc("TRN2", target_bir_lowering=False, debug=False)
    build_kernel(nc, steps, repeat)
    nc.compile()
    return nc


def shard_inputs(inputs, steps: int):
    """Full inputs dict -> list of 8 per-core input maps."""
    B = inputs["y0"].shape[0]
    nb = B // P
    maps = []
    fa = lambda x: np.ascontiguousarray(np.asarray(x, dtype=np.float32))
    for i in range(nb):
        s = slice(i * P, (i + 1) * P)
        m = {
            "y0": fa(inputs["y0"][s]),
            "x_future": fa(inputs["x_future"][s, :steps].reshape(P, steps * EXO)),
            "h00": fa(inputs["h0"][0, s]),
            "h01": fa(inputs["h0"][1, s]),
            "c00": fa(inputs["c0"][0, s]),
            "c01": fa(inputs["c0"][1, s]),
            "z": fa(inputs["z"][s]),
            "W_ih0": fa(inputs["W_ih0"]),
            "W_hh0": fa(inputs["W_hh0"]),
            "b_ih0": fa(inputs["b_ih0"]).reshape(1, G),
            "b_hh0": fa(inputs["b_hh0"]).reshape(1, G),
            "W_ih1": fa(inputs["W_ih1"]),
            "W_hh1": fa(inputs["W_hh1"]),
            "b_ih1": fa(inputs["b_ih1"]).reshape(1, G),
            "b_hh1": fa(inputs["b_hh1"]).reshape(1, G),
            "W_proj": fa(inputs["W_proj"]),
            "b_proj": fa(inputs["b_proj"]).reshape(1, 1),
            "W_z": fa(inputs["W_z"]),
            "b_z": fa(inputs["b_z"]).reshape(1, 9),
        }
        maps.append(m)
    return maps


def assemble_output(results, steps: int):
    """list of per-core {"out": [P, S]} -> [B, S, 1]."""
    outs = [np.ascontiguousarray(np.asarray(rm["out"]).T).reshape(P, steps, 1) for rm in results]
    return np.concatenate(outs, axis=0)


# ======================= public entry point =======================
_NC_CACHE = {}


def _get_nc():
    if "nc" not in _NC_CACHE:
        _NC_CACHE["nc"] = make_nc(STEPS)
    return _NC_CACHE["nc"]


STEPS = 64
N_CORES = 8


def kernel(**inputs):
    """Full-input entry point: shards batch over 8 NeuronCores, runs the
    Bass LSTM-decoder kernel, reassembles [B, steps, 1] float32 output."""
    from concourse.bass_utils import run_bass_kernel_spmd

    steps = int(inputs.get("steps", STEPS))
    assert steps == STEPS, f"kernel compiled for {STEPS} steps, got {steps}"
    nc = _get_nc()
    maps = shard_inputs(inputs, STEPS)
    res = run_bass_kernel_spmd(nc, maps, list(range(N_CORES)))
    return assemble_output(res.results, STEPS).astype(np.float32)



# revision 36
# speedup vs baseline: 1.8459x; 1.8459x over previous
"""2-layer LSTM decoder Bass/Tile kernel for TRN2 — transposed layout, bf16.

Per-core: B_local=128 batch rows, H=512, 64 steps. Data-parallel over 8
cores; weights replicated (host pre-transposes weights into lhsT layout).

Layout "T": features on partitions, batch on the free dim.
  gates.T chunk [128 gate-rows, 128 batch] = W_chunk.T @ hT_chunk
  matmul(out=gates_chunk, lhsT=W_slice[128 hid, 128 gate], rhs=hT[128 hid, 128 b])
All state (h, c, sigmoid/tanh outputs) stays in the transposed layout, so the
recurrence needs NO PE transposes; gate biases ride as extra contraction rows
(L0: ones row in the K=10 input matmul; L1: one K=1 matmul per gate chunk).
Weights/h in bf16 (1 cycle/row matmuls at N=128), c and gate accumulation f32.

PSUM chunk map (per layer, [128, 2048] f32 = 4 banks):
  i -> [0:512], f -> [512:1024], o -> [1024:1536], g -> [1536:2048]
so one sigmoid ACT covers a contiguous span and tanh another.
"""

import numpy as np
from contextlib import ExitStack

import concourse.bass as bass
import concourse.bacc as bacc
import concourse.mybir as mybir
import concourse.tile as tile

F32 = mybir.dt.float32
F32R = mybir.dt.float32r
BF16 = mybir.dt.bfloat16
AF = mybir.ActivationFunctionType
OP = mybir.AluOpType

P = 128           # batch rows per core
H = 512           # hidden
G = 2048          # 4*H gates
KC = 4            # K chunks of 128 across H
NCH = 16          # gate chunks of 128 across G
S = 64            # steps
EXO = 8
ZD = 16

# pytorch gate order by chunk: i: 0-3, f: 4-7, g: 8-11, o: 12-15
# PSUM free-offset: i->[0:512], f->[512:1024], o->[1024:1536], g->[1536:2048]
def chunk_off(c):
    if c < 8:
        return c * P
    if c < 12:
        return 1536 + (c - 8) * P
    return 1024 + (c - 12) * P


# emission order: g chunks first (tanh ACT fires early), then i, f, o so the
# split sigmoid calls fire as their chunks complete
CH_ORDER = [8, 9, 10, 11, 0, 1, 2, 3, 4, 5, 6, 7, 12, 13, 14, 15]
# stop-MM order for the group-closing passes (B and D): i first so sg_i can
# start while f/o chunks are still in the matmul stream, then g (tanh), f, o
CH_STOP = [0, 1, 2, 3, 8, 9, 10, 11, 4, 5, 6, 7, 12, 13, 14, 15]
NA1 = 12  # A-chunks (g,i,f) emitted between D and the proj matmuls; o-chunks after


ROLES = {}  # instruction name -> role string (diagnostics only)


def _tag_role(inst, role):
    try:
        ROLES[inst.ins.name] = role
    except Exception:
        pass
    return inst


def build_kernel(nc: bass.Bass, steps: int):
    assert steps == S
    # ---- DRAM params (host-prepped layouts; see shard_inputs) ----
    def di(name, shape, dt):
        return nc.dram_tensor(name, shape, dt, kind="ExternalInput").ap()

    whh0T = di("whh0T", [P, KC * G], BF16)   # [p, k*G+g] = W_hh0[g, k*128+p]
    wih1T = di("wih1T", [P, KC * G], BF16)
    whh1T = di("whh1T", [P, KC * G], BF16)
    wa0 = di("wa0", [10, G], BF16)           # rows: 0 y-col, 1-8 exo cols, 9 zeros
    bih0 = di("b_ih0", [8, G // 8], F32)
    bhh0 = di("b_hh0", [8, G // 8], F32)
    bih1 = di("b_ih1", [8, G // 8], F32)
    bhh1 = di("b_hh1", [8, G // 8], F32)
    wpT = di("wpT", [P, KC], F32)            # [p, k] = W_proj[0, k*128+p]
    bp = di("bp", [1, 1], F32)
    wzT = di("wzT", [ZD, 9], F32)            # W_z.T
    bz8 = di("bz8", [8, 1], F32)             # b_z[1:9]
    bz0 = di("bz0", [1, 1], F32)             # b_z[0]
    zT = di("zT", [ZD, P], F32)              # z.T
    y0T = di("y0T", [1, P], F32)
    xfT = di("xfT", [EXO, S * P], BF16)      # [e, t*128+b] = x_future[b, t, e]
    h0Td = di("h0T", [P, H], BF16)           # [p, k*128+b] = h0[b, k*128+p]
    h1Td = di("h1T", [P, H], BF16)
    c0Td = di("c0T", [P, H], F32)
    c1Td = di("c1T", [P, H], F32)
    out_d = nc.dram_tensor("out", [S, P], F32, kind="ExternalOutput").ap()

    with tile.TileContext(nc) as tc, ExitStack() as ctx:
        emit(ctx, tc, nc, locals())
    return nc


def emit(ctx, tc, nc, t_in):
    whh0T, wih1T, whh1T = t_in["whh0T"], t_in["wih1T"], t_in["whh1T"]
    wa0, wpT, bp = t_in["wa0"], t_in["wpT"], t_in["bp"]
    bih0, bhh0, bih1, bhh1 = t_in["bih0"], t_in["bhh0"], t_in["bih1"], t_in["bhh1"]
    wzT, zT, y0T, xfT = t_in["wzT"], t_in["zT"], t_in["y0T"], t_in["xfT"]
    bz8, bz0 = t_in["bz8"], t_in["bz0"]
    h0Td, h1Td, c0Td, c1Td = t_in["h0Td"], t_in["h1Td"], t_in["c0Td"], t_in["c1Td"]
    out_d = t_in["out_d"]

    # ---- pools ----
    const = ctx.enter_context(tc.tile_pool(name="const", bufs=1))
    ldtmp = ctx.enter_context(tc.tile_pool(name="ldtmp", bufs=1))
    state = ctx.enter_context(tc.tile_pool(name="state", bufs=2))
    act = ctx.enter_context(tc.tile_pool(name="act", bufs=2))
    yo = ctx.enter_context(tc.tile_pool(name="yo", bufs=3))
    g0p = ctx.enter_context(tc.tile_pool(name="g0p", bufs=1, space="PSUM"))
    g1p = ctx.enter_context(tc.tile_pool(name="g1p", bufs=1, space="PSUM"))

    dma = nc.sync.dma_start
    adma = nc.scalar.dma_start
    gdma = nc.gpsimd.dma_start

    # ---- persistent SBUF ----
    Wh0 = const.tile([P, KC * G], BF16, name="Wh0")
    Wi1 = const.tile([P, KC * G], BF16, name="Wi1")
    Wh1 = const.tile([P, KC * G], BF16, name="Wh1")
    wa0_sb = const.tile([10, G], BF16, name="wa0_sb")
    b1c = const.tile([1, G], BF16, name="b1c")
    wpT_sb = const.tile([P, KC], BF16, name="wpT_sb")
    inT = const.tile([10, S * P], BF16, name="inT")  # p0 y, p1-8 exo, p9 ones
    zb8 = const.tile([8, P], BF16, name="zb8")     # z-bias for exo rows
    zby = const.tile([1, P], F32, name="zby")      # z-bias for the y row
    bz8_sb = const.tile([8, 1], F32, name="bz8_sb")
    bz0_sb = const.tile([1, 1], F32, name="bz0_sb")
    bp_row = const.tile([1, P], F32, name="bp_row")
    yrow_bias = const.tile([1, P], F32, name="yrow_bias")  # bp + zb[0]
    ones_bf = const.tile([1, P], BF16, name="ones_bf")

    # ---- init loads (queue order matters: states + Wh0 gate step 0) ----
    # SP queue: states, then Wh0, then wa0
    h0T_c = state.tile([P, H], BF16, name="h0T", tag="h0T")
    h1T_c = state.tile([P, H], BF16, name="h1T", tag="h1T")
    c0_c = state.tile([P, H], F32, name="c0", tag="c0")
    c1_c = state.tile([P, H], F32, name="c1", tag="c1")
    dma(h0T_c[:], h0Td)
    dma(h1T_c[:], h1Td)
    dma(c0_c[:], c0Td)
    dma(c1_c[:], c1Td)
    dma(Wh0[:], whh0T)
    dma(wa0_sb[:], wa0)
    # gpsimd queue: z tensors (gate the zb matmuls), Wh1, then xfT
    wzT_sb = ldtmp.tile([ZD, 9], F32R, name="wzT_sb", tag="wz")
    gdma(wzT_sb[:], wzT)
    zT_sb = ldtmp.tile([ZD, P], F32R, name="zT_sb", tag="zt")
    gdma(zT_sb[:], zT)
    xfT_sb = ldtmp.tile([EXO, S * P], BF16, name="xfT_sb", tag="xf")
    gdma(xfT_sb[:], xfT)
    gdma(Wi1[:], wih1T)
    gdma(Wh1[:], whh1T)
    # ACT queue: dummy sigmoid first so the table set loads immediately,
    # then the small admas that gate the zb ACTs
    dumm = ldtmp.tile([1, 1], F32, name="dumm", tag="dumm")
    nc.vector.memset(dumm[:], 0.0)
    nc.scalar.activation(dumm[:], dumm[:], AF.Sigmoid)
    adma(bz8_sb[:], bz8)
    adma(bz0_sb[:], bz0)
    y0T_sb = ldtmp.tile([1, P], F32, name="y0T_sb", tag="y0")
    adma(y0T_sb[:], y0T)

    nc.vector.memset(ones_bf[:], 1.0)

    # ---- z bias: zb8[8, P] = W_z[1:9] @ z.T + b_z[1:9]; zby = row 0 ----
    # (engine APs must start at a 32-aligned partition, so the y-bias row is
    # computed as its own base-0 matmul instead of slicing partition 8)
    zb_ps = g0p.tile([P, H], F32, name="zbps", tag="g0i")
    nc.tensor.matmul(
        zb_ps[0:8, 0:P], wzT_sb[:, 1:9], zT_sb[:],
        start=True, stop=True,
    )
    nc.tensor.matmul(
        zb_ps[0:1, P : 2 * P], wzT_sb[:, 0:1], zT_sb[:],
        start=True, stop=True,
    )
    nc.scalar.activation(zb8[:], zb_ps[0:8, 0:P], AF.Identity, bias=bz8_sb[:])
    nc.scalar.activation(zby[:], zb_ps[0:1, P : 2 * P], AF.Identity, bias=bz0_sb[:])

    # rest of the ACT-queue loads (after the zb ACTs so they don't gate them)
    bp_sb = ldtmp.tile([1, 1], F32, name="bp_sb", tag="bp")
    adma(bp_sb[:], bp)
    wpf = ldtmp.tile([P, KC], F32, name="wpf", tag="wp")
    adma(wpf[:], wpT)
    nc.vector.tensor_copy(wpT_sb[:], wpf[:])  # f32 -> bf16
    bih0_sb = ldtmp.tile([8, G // 8], F32, name="bih0_sb", tag="b0a")
    bhh0_sb = ldtmp.tile([8, G // 8], F32, name="bhh0_sb", tag="b0b")
    bih1_sb = ldtmp.tile([8, G // 8], F32, name="bih1_sb", tag="b1a")
    bhh1_sb = ldtmp.tile([8, G // 8], F32, name="bhh1_sb", tag="b1b")
    adma(bih0_sb[:], bih0)
    adma(bhh0_sb[:], bhh0)
    adma(bih1_sb[:], bih1)
    adma(bhh1_sb[:], bhh1)

    # exo rows with z-bias baked in, built in a base-0 staging tile then
    # DMA'd to partitions 1-9 of inT (engine APs can't start at partition 1).
    # Row 8 of the staging tile stays 1.0 (the ones row for the b0 bias).
    stage = ldtmp.tile([9, S * P], BF16, name="stage", tag="stage")
    nc.vector.memset(stage[:], 1.0)
    SPLIT = 8
    x3 = xfT_sb.rearrange("e (t b) -> e t b", b=P)
    dst3 = stage[0:8, :].rearrange("e (t b) -> e t b", b=P)
    zb3a = zb8.unsqueeze(1).broadcast_to((EXO, SPLIT, P))
    zb3b = zb8.unsqueeze(1).broadcast_to((EXO, S - SPLIT, P))
    nc.vector.tensor_tensor(dst3[:, 0:SPLIT], x3[:, 0:SPLIT], zb3a, op=OP.add)
    dma(inT[1:9, 0 : SPLIT * P], stage[0:8, 0 : SPLIT * P])
    nc.vector.tensor_tensor(dst3[:, SPLIT:S], x3[:, SPLIT:S], zb3b, op=OP.add)
    dma(inT[1:9, SPLIT * P :], stage[0:8, SPLIT * P :])
    adma(inT[9:10, :], stage[8:9, :])
    # y row for t=0
    nc.vector.tensor_tensor(inT[0:1, 0:P], y0T_sb[:], zby[:], op=OP.add)
    # combined biases -> bf16 rows (built [8, 256], DMA-reshaped into place)
    b0st = ldtmp.tile([8, G // 8], BF16, name="b0st", tag="b0st")
    nc.vector.tensor_tensor(b0st[:], bih0_sb[:], bhh0_sb[:], op=OP.add)
    adma(wa0_sb[9:10, :], b0st[:])
    b1st = ldtmp.tile([8, G // 8], BF16, name="b1st", tag="b1st")
    nc.vector.tensor_tensor(b1st[:], bih1_sb[:], bhh1_sb[:], op=OP.add)
    adma(b1c[:], b1st[:])
    # bp rows
    nc.vector.tensor_copy(bp_row[:], bp_sb[0:1, 0:1].broadcast_to((1, P)))
    nc.vector.tensor_tensor(yrow_bias[:], zby[:], bp_row[:], op=OP.add)

    # =========================== STEP LOOP ===========================
    CUR = ["init"]

    def R(role):
        CUR[0] = role

    def mm(*a, **k):
        return _tag_role(nc.tensor.matmul(*a, **k), CUR[0])

    def sact(*a, **k):
        return _tag_role(nc.scalar.activation(*a, **k), CUR[0])

    def vtt(*a, **k):
        return _tag_role(nc.vector.tensor_tensor(*a, **k), CUR[0])

    # Gates live in FOUR separate 1-bank PSUM tiles per layer (one per gate
    # type) so the tail ACT reads never alias the other types' matmul writes
    # (PSUM dependency tracking is tile-granular; a shared 4-bank tile chains
    # every stop group behind the previous ACT read).
    TYPE = "ifgo"  # chunk c//4 -> tile key

    def g_tiles(pool, pfx):
        return {t: pool.tile([P, H], F32, name=pfx + t, tag=pfx + t) for t in TYPE}

    def dst_of(g, c):
        return g[TYPE[c // 4]][:, (c % 4) * P : (c % 4 + 1) * P]

    def emit_A(g0, h0T, chunks, started):
        """L0 hidden matmuls. PSUM start=True zeroes a whole 2KB bank
        ("zero region"), so exactly ONE group per tile per step: start only
        on the first matmul that touches the tile, stop on the last B MM."""
        R("A")
        for c in chunks:
            dst = dst_of(g0, c)
            ty = TYPE[c // 4]
            for k in range(KC):
                mm(
                    dst, Wh0[:, k * G + c * P : k * G + (c + 1) * P],
                    h0T[:, k * P : (k + 1) * P],
                    start=not started.get(ty, False), stop=False,
                )
                started[ty] = True

    STOP_GROUPS = [[0, 1, 2, 3], [8, 9, 10, 11], [4, 5, 6, 7], [12, 13, 14, 15]]

    def emit_B(g0, t, parts):
        """L0 input matmuls (K=10: y, exo, ones->bias); the tile's last MM
        carries the group stop. Tail ACT/DVE ops are emitted between chunk
        groups so each sigmoid/tanh syncs on its own gates."""
        rhs = inT[:, t * P : (t + 1) * P]
        for grp, part in zip(STOP_GROUPS, parts):
            R("B")
            for j, c in enumerate(grp):
                mm(
                    dst_of(g0, c), wa0_sb[:, c * P : (c + 1) * P], rhs,
                    start=False, stop=(j == len(grp) - 1),
                )
            part()

    def emit_C(g1, h1T, chunks, started):
        """L1 h1-part matmuls + bias rows (neither needs h0). One group per
        tile: start only on the tile's first matmul."""
        R("C")
        for c in chunks:
            dst = dst_of(g1, c)
            ty = TYPE[c // 4]
            for k in range(KC):
                mm(
                    dst, Wh1[:, k * G + c * P : k * G + (c + 1) * P],
                    h1T[:, k * P : (k + 1) * P],
                    start=not started.get(ty, False), stop=False,
                )
                started[ty] = True
            mm(
                dst, b1c[:, c * P : (c + 1) * P], ones_bf[:],
                start=False, stop=False,
            )

    def emit_D(g1, h0T, parts):
        """L1 h0-part matmuls, i chunks first; the tile's last MM carries
        the group stop; tail ACT/DVE ops interleave between chunk groups."""
        for grp, part in zip(STOP_GROUPS, parts):
            R("D")
            for j, c in enumerate(grp):
                dst = dst_of(g1, c)
                for k in range(KC):
                    mm(
                        dst, Wi1[:, k * G + c * P : k * G + (c + 1) * P],
                        h0T[:, k * P : (k + 1) * P], start=False,
                        stop=(j == len(grp) - 1 and k == KC - 1),
                    )
            part()

    def make_tail(g, c_prev, htag, ctag):
        """sig/tanh + c/h update in transposed layout, as 4 emit-callbacks
        (one per stop group: i, g, f, o) to interleave with the gate matmul
        stream. All elementwise on DVE (the v2 cost model charges Pool
        TensorTensor ~2.2 ns/elem + launch, DVE is ~1.04)."""
        sg_i = act.tile([P, H], F32, name="sg_i", tag="sg_i")
        tg = act.tile([P, H], F32, name="tg", tag="tg")
        sg_f = act.tile([P, H], F32, name="sg_f", tag="sg_f")
        sg_o = act.tile([P, H], F32, name="sg_o", tag="sg_o")
        t1 = act.tile([P, H], F32, name="t1", tag="t1")
        t2 = act.tile([P, H], F32, name="t2", tag="t2")
        thc = act.tile([P, H], F32, name="thc", tag="thc")
        c_n = state.tile([P, H], F32, name=ctag, tag=ctag)
        h_n = state.tile([P, H], BF16, name=htag, tag=htag)

        def p_i():
            R(htag + ":sg_i")
            sact(sg_i[:], g["i"][:], AF.Sigmoid)

        HH = H // 2
        ha, hb = slice(0, HH), slice(HH, H)

        def p_g():
            R(htag + ":tg")
            sact(tg[:], g["g"][:], AF.Tanh)
            vtt(t1[:, ha], sg_i[:, ha], tg[:, ha], op=OP.mult)
            vtt(t1[:, hb], sg_i[:, hb], tg[:, hb], op=OP.mult)

        def p_f():
            R(htag + ":sg_f")
            sact(sg_f[:], g["f"][:], AF.Sigmoid)
            vtt(t2[:, ha], sg_f[:, ha], c_prev[:, ha], op=OP.mult)
            vtt(c_n[:, ha], t1[:, ha], t2[:, ha], op=OP.add)
            vtt(t2[:, hb], sg_f[:, hb], c_prev[:, hb], op=OP.mult)
            vtt(c_n[:, hb], t1[:, hb], t2[:, hb], op=OP.add)

        def p_o():
            R(htag + ":sg_o")
            sact(sg_o[:], g["o"][:], AF.Sigmoid)
            sact(thc[:, ha], c_n[:, ha], AF.Tanh)
            vtt(h_n[:, ha], sg_o[:, ha], thc[:, ha], op=OP.mult)
            sact(thc[:, hb], c_n[:, hb], AF.Tanh)
            vtt(h_n[:, hb], sg_o[:, hb], thc[:, hb], op=OP.mult)

        return [p_i, p_g, p_f, p_o], c_n, h_n

    # prologue: step-0 L0 gates (+interleaved tail) and L1 h1-part
    g0_cur = g_tiles(g0p, "g0")
    emit_A(g0_cur, h0T_c, CH_ORDER, {})
    parts0, c0_n, h0T_cur = make_tail(g0_cur, c0_c, "h0T", "c0")
    emit_B(g0_cur, 0, parts0)
    c0_c = c0_n
    g1_cur = g_tiles(g1p, "g1")
    emit_C(g1_cur, h1T_c, CH_ORDER, {})

    for t in range(S):
        # ---- PE: L1 h0-part + bias for t, interleaved with the L1 tail ----
        parts1, c1_n, h1T_n = make_tail(g1_cur, c1_c, "h1T", "c1")
        emit_D(g1_cur, h0T_cur, parts1)
        c1_c = c1_n
        # ---- PE: L0 hidden for t+1, first part (covers L1-tail latency) ----
        if t + 1 < S:
            g0_nxt = g_tiles(g0p, "g0")
            a_started = {}
            emit_A(g0_nxt, h0T_cur, CH_ORDER[:NA1], a_started)
        # ---- PE: projection yT(t) = Wp @ h1(t) ----
        # Writes into the last o-chunk's region of g1's o-tile (already
        # consumed by sg_o); start=True re-claims the PSUM words, so no
        # extra bank is needed.
        pj = g1_cur["o"][0:1, 3 * P : 4 * P]
        R("proj")
        for k in range(KC):
            mm(
                pj, wpT_sb[:, k : k + 1], h1T_n[:, k * P : (k + 1) * P],
                start=(k == 0), stop=(k == KC - 1),
            )
        # ---- y outputs ----
        yo_t = yo.tile([1, P], F32, name="yout", tag="yout")
        R("y_out")
        vtt(yo_t[:], pj, bp_row[:], op=OP.add)
        dma(out_d[t : t + 1, :], yo_t[:])
        if t + 1 < S:
            # next-step y input row (yhat + zb[0], cast to bf16)
            R("y_row")
            vtt(
                inT[0:1, (t + 1) * P : (t + 2) * P], pj, yrow_bias[:],
                op=OP.add,
            )
            # o-chunk starts (cover the y latency), then the y-gated input
            # stops with the L0 tail interleaved
            emit_A(g0_nxt, h0T_cur, CH_ORDER[NA1:], a_started)
            parts0, c0_n, h0T_nxt = make_tail(g0_nxt, c0_c, "h0T", "c0")
            emit_B(g0_nxt, t + 1, parts0)
            c0_c, h0T_cur = c0_n, h0T_nxt
            g1_nxt = g_tiles(g1p, "g1")
            emit_C(g1_nxt, h1T_n, CH_ORDER, {})
            g0_cur, g1_cur = g0_nxt, g1_nxt


def make_nc(steps: int = S):
    nc = bacc.Bacc("TRN2", target_bir_lowering=False, debug=False)
    build_kernel(nc, steps)
    nc.compile()
    return nc


# ======================= host-side prep =======================

def _bf16(x):
    import ml_dtypes
    return np.ascontiguousarray(x.astype(ml_dtypes.bfloat16))


def _f32(x):
    return np.ascontiguousarray(np.asarray(x, dtype=np.float32))


def _wT4(W):
    """[G, H] f32 -> [128, KC*G] bf16 with [p, k*G+g] = W[g, k*128+p]."""
    W = np.asarray(W, dtype=np.float32)
    Wt = W.T.reshape(KC, P, G).transpose(1, 0, 2).reshape(P, KC * G)
    return _bf16(Wt)


def _hT4(h):
    """[B_loc, H] -> [128, H] with [p, k*128+b] = h[b, k*128+p]."""
    return h.T.reshape(KC, P, P).transpose(1, 0, 2).reshape(P, H)


def shard_inputs(inputs, steps: int):
    B = inputs["y0"].shape[0]
    nb = B // P
    # replicated (host layout prep done once)
    wa0 = np.concatenate(
        [
            np.asarray(inputs["W_ih0"], np.float32)[:, 0:1].T,     # y col
            np.asarray(inputs["W_ih0"], np.float32)[:, 1:9].T,     # exo cols
            np.zeros((1, G), np.float32),                          # b0 filled on device
        ],
        axis=0,
    )
    rep = {
        "whh0T": _wT4(inputs["W_hh0"]),
        "wih1T": _wT4(inputs["W_ih1"]),
        "whh1T": _wT4(inputs["W_hh1"]),
        "wa0": _bf16(wa0),
        "b_ih0": _f32(inputs["b_ih0"]).reshape(8, G // 8),
        "b_hh0": _f32(inputs["b_hh0"]).reshape(8, G // 8),
        "b_ih1": _f32(inputs["b_ih1"]).reshape(8, G // 8),
        "b_hh1": _f32(inputs["b_hh1"]).reshape(8, G // 8),
        "wpT": _f32(inputs["W_proj"]).reshape(KC, P).T.copy(),
        "bp": _f32(inputs["b_proj"]).reshape(1, 1),
        "wzT": _f32(inputs["W_z"]).T.copy(),
        "bz8": _f32(inputs["b_z"]).reshape(9, 1)[1:9].copy(),
        "bz0": _f32(inputs["b_z"]).reshape(9, 1)[0:1].copy(),
    }
    maps = []
    for i in range(nb):
        s = slice(i * P, (i + 1) * P)
        x = np.asarray(inputs["x_future"], np.float32)[s, :steps]  # [P, S, E]
        m = dict(rep)
        m.update(
            {
                "zT": _f32(inputs["z"][s]).T.copy(),
                "y0T": _f32(inputs["y0"][s]).reshape(1, P).copy(),
                "xfT": _bf16(x.transpose(2, 1, 0).reshape(EXO, steps * P).astype(np.float32)),
                "h0T": _bf16(_hT4(np.asarray(inputs["h0"], np.float32)[0, s])),
                "h1T": _bf16(_hT4(np.asarray(inputs["h0"], np.float32)[1, s])),
                "c0T": _f32(_hT4(np.asarray(inputs["c0"], np.float32)[0, s])),
                "c1T": _f32(_hT4(np.asarray(inputs["c0"], np.float32)[1, s])),
            }
        )
        maps.append(m)
    return maps


def assemble_output(results, steps: int):
    outs = [
        np.ascontiguousarray(np.asarray(rm["out"]).T).reshape(P, steps, 1)
        for rm in results
    ]
    return np.concatenate(outs, axis=0)


# ======================= public entry point =======================
_NC_CACHE = {}


def _get_nc():
    if "nc" not in _NC_CACHE:
        _NC_CACHE["nc"] = make_nc(S)
    return _NC_CACHE["nc"]


STEPS = S
N_CORES = 8


def kernel(**inputs):
    """Full-input entry point: shards batch over 8 NeuronCores, runs the
    Bass LSTM-decoder kernel, reassembles [B, steps, 1] float32 output."""
    from concourse.bass_utils import run_bass_kernel_spmd

    steps = int(inputs.get("steps", STEPS))
    assert steps == STEPS, f"kernel compiled for {STEPS} steps, got {steps}"
    nc = _get_nc()
    maps = shard_inputs(inputs, STEPS)
    res = run_bass_kernel_spmd(nc, maps, list(range(N_CORES)))
    return assemble_output(res.results, STEPS).astype(np.float32)
